# revision 15
# baseline (speedup 1.0000x reference)
"""DRAW-model Trainium2 kernel (8 NeuronCores, data-parallel over batch).

Strategy
--------
Pure data parallelism: 8 cores x 64 local batch, zero collectives.  All
weights SBUF-resident fp16; activations feature-minor ("transposed") so the
big encoder matmul streams N=512-wide moving operands at full PE rate.

v2 changes over the original baseline:
  * all sigmoids expressed as tanh (sig(x) = (1+tanh(x/2))/2) with the
    0.5 factors folded into host-side weights (cell state kept as c2=2c,
    hidden state as h2=2h with halved downstream weights).  Every ACT
    function used (Tanh, Exp) then lives in the single `exp_and_others`
    activation table -> zero per-step ACT_TABLE_LOADs (was 2 x 1.3us).
  * gate columns permuted to [i/2 | g] ++ [f/2 | o/2] so each col-tiled
    matmul half needs exactly one N=512 tanh; the cross-half join
    (c2' = v/2 + u) uses a PE identity-shift matmul instead of the
    per-step SBUF->SBUF DMA.
  * attention reformulated batch-major: logits [64,10] -> Exp with
    accum_out (fused row-sum) -> reciprocal -> per-partition scale ->
    one PE transpose.  The decoder consumes attn directly through the
    host-folded Wz = W_enc^T @ dec_kernel [10,1024], killing the old
    z-materialization (2 matmuls + broadcast matmul + 2 vector muls).
  * decoder matmul col-tiled like the encoder (concurrent halves).
  * dependency-anchored LDWEIGHTS heartbeats inside the gate chains keep
    the PE HAM clock-gate at K=8/8 (the old kernel re-throttled to
    1.2 GHz every step during the ~5us scalar window, running the
    decoder+canvas matmuls at half clock).
  * PSUM evacuation copies moved from the Scalar to the Vector engine.
"""

import numpy as np

STEPS = 10
UNITS = 256
BL = 64          # local batch per core
NCORES = 8
IMG = 64

# ---------------------------------------------------------------- host index math
def _pix_order():
    # new pixel index n = q*1024 + j*32 + i  ->  original pixel (2j+pr)*64 + (2i+pc)
    # with q = pr*2 + pc
    out = np.empty(4096, np.int64)
    n = 0
    for pr in range(2):
        for pc in range(2):
            for j in range(32):
                for i in range(32):
                    out[n] = (2 * j + pr) * 64 + (2 * i + pc)
                    n += 1
    return out


PIX = _pix_order()


def fold_enc_kernel(W):
    """Collapse extract_patches into the weight: each patch feature copies one
    pixel of x_hat, so patches @ W[:9216] == x_hat_flat @ A with
    A[p,:] = sum of W rows whose feature reads pixel p.  [4096, 1024]."""
    A = np.zeros((4096, W.shape[1]), np.float32)
    r_idx = np.arange(32)
    for dy in range(3):
        rows = 2 * r_idx + dy
        rv = r_idx[rows < 64]
        for dx in range(3):
            cols = 2 * r_idx + dx
            cv = r_idx[cols < 64]
            pix = (2 * rv[:, None] + dy) * 64 + (2 * cv[None, :] + dx)
            feat = (rv[:, None] * 32 + cv[None, :]) * 9 + (dy * 3 + dx)
            A[pix.ravel()] += W[feat.ravel()]
    return A


def perm_scale(W):
    """Permute gate columns i,f,g,o -> [i/2, g, f/2, o/2] so that the
    (0,0) col-tile half computes [z_i/2 | z_g] and the (0,64) half
    [z_f/2 | z_o/2]; tanh of those gives ti,tg,tf,to directly."""
    return np.concatenate([W[:, 0:256] * 0.5, W[:, 512:768],
                           W[:, 256:512] * 0.5, W[:, 768:1024] * 0.5], axis=1)


# ---------------------------------------------------------------- program builder
def build_program(steps=STEPS, has_bdec=False):
    """Build + compile the per-core Bass program.  Returns nc."""
    import concourse.bacc as bacc
    import concourse.tile as tile
    import concourse.mybir as mybir
    from concourse.alu_op_type import AluOpType as Op

    f16 = mybir.dt.float16
    f32 = mybir.dt.float32
    AF = mybir.ActivationFunctionType

    nc = bacc.Bacc("TRN2", target_bir_lowering=False, debug=False,
                   dynamic_dma_scratch_size=2048)

    def din(name, shape, dt):
        return nc.dram_tensor(name, shape, dt, kind="ExternalInput")

    d_encw = din("encw", [128, 32 * 1024], f16)
    d_recw = din("recw", [128, 5 * 1024], f16)
    d_decw = din("decw", [128, 4 * 1024], f16)
    d_wdec = din("wdec", [128, 2 * 4096], f16)
    d_wenc = din("wenc", [128, 32], f16)
    d_bencr = din("bencr", [128, 16], f16)
    d_iden = din("iden", [128, 64], f16)
    d_onesst = din("onesst", [128, 64], f16)
    d_xm1 = din("xm1", [128, 2048], f16)
    if has_bdec:
        d_bdec = din("bdec", [1, 4096], f16)
    d_out = nc.dram_tensor("canvas", [128, 2048], f32, kind="ExternalOutput")

    with tile.TileContext(nc) as tc:
        # ---------------- static SBUF
        s_encw = nc.alloc_sbuf_tensor("s_encw", [128, 32, 1024], f16)
        s_recw = nc.alloc_sbuf_tensor("s_recw", [128, 5, 1024], f16)
        s_decw = nc.alloc_sbuf_tensor("s_decw", [128, 4, 1024], f16)
        s_wdec = nc.alloc_sbuf_tensor("s_wdec", [128, 2, 4096], f16)
        s_wenc = nc.alloc_sbuf_tensor("s_wenc", [128, 2, 16], f16)
        s_bencr = nc.alloc_sbuf_tensor("s_bencr", [128, 16], f16)
        s_iden = nc.alloc_sbuf_tensor("s_iden", [128, 64], f16)
        s_onesst = nc.alloc_sbuf_tensor("s_onesst", [128, 64], f16)
        s_xm1 = nc.alloc_sbuf_tensor("s_xm1", [128, 4, 8, 64], f16)
        s_xhat = nc.alloc_sbuf_tensor("s_xhat", [128, 4, 8, 64], f16)
        s_hencT = nc.alloc_sbuf_tensor("s_hencT", [128, 2, 64], f16)
        s_hdecT = nc.alloc_sbuf_tensor("s_hdecT", [128, 2, 64], f16)
        s_attnT = nc.alloc_sbuf_tensor("s_attnT", [16, 64], f16)
        # cell states live on partitions 64-127 (with the f|o gate half)
        s_cenc = nc.alloc_sbuf_tensor("s_cenc", [128, 256], f32)
        s_cdec = nc.alloc_sbuf_tensor("s_cdec", [128, 256], f32)
        if has_bdec:
            s_bdec = nc.alloc_sbuf_tensor("s_bdec", [1, 4096], f16)

        # ---------------- load weights / constants (two parallel DMA queues:
        # sync carries xm1 + encw in consumption order; scalar carries the
        # small weights + decoder-side tensors needed mid-step-0)
        recw_d_ap = d_recw.ap().rearrange("p (t n) -> p t n", n=1024)
        for dst, src in [
            (s_onesst[:, :], d_onesst.ap()),
            (s_xm1[:, :, :, :], d_xm1.ap()),
        ]:
            nc.sync.dma_start(out=dst, in_=src)
        for dst, src in [
            (s_recw[:, 4, :], recw_d_ap[:, 4, :]),   # bias row: 1st matmul dep
            (s_iden[:, :], d_iden.ap()),
            (s_recw[:, 0:4, :], recw_d_ap[:, 0:4, :]),
            (s_wenc[:, :, :], d_wenc.ap()), (s_bencr[:, :], d_bencr.ap()),
            (s_decw[:, :, :], d_decw.ap()), (s_wdec[:, :, :], d_wdec.ap()),
        ]:
            nc.scalar.dma_start(out=dst, in_=src)
        encw_d_ap = d_encw.ap().rearrange("p (t n) -> p t n", n=1024)
        for g in range(16):
            sl = slice(g * 2, (g + 1) * 2)
            nc.sync.dma_start(out=s_encw[:, sl, :], in_=encw_d_ap[:, sl, :])
        if has_bdec:
            nc.scalar.dma_start(out=s_bdec[:, :], in_=d_bdec.ap())

        # ---------------- pools
        import contextlib
        ctx = contextlib.ExitStack()
        work = ctx.enter_context(tc.tile_pool(name="work", bufs=2))
        p_ig = ctx.enter_context(tc.tile_pool(name="p_ig", bufs=1, space="PSUM"))
        p_fo = ctx.enter_context(tc.tile_pool(name="p_fo", bufs=1, space="PSUM"))
        p_sm = ctx.enter_context(tc.tile_pool(name="p_sm", bufs=2, space="PSUM"))
        p_cv = ctx.enter_context(tc.tile_pool(name="p_cv", bufs=1, space="PSUM"))

        def gates(ps_ig, ps_fo, c_s, hT_dst, first):
            """Tanh-only LSTM gate math, pipelined in two 128-col halves.
            ps_ig: [64,512] PSUM AP holding [z_i/2 | z_g] on parts 0-63;
            ps_fo: [64,512] AP on parts 64-127 holding [z_f/2 | z_o/2].
            c_s: [64,256] f32 SBUF at parts 64-127 (c2 = 2c).  Writes
            transposed fp16 h2 into hT_dst."""
            tig = work.tile([128, 512], f16, tag="tig")
            tfo = work.tile([128, 512], f16, tag="tfo")
            nc.scalar.activation(tig[0:64, :], ps_ig, AF.Tanh)
            # warm-keeper: dependency-anchored PE work so the HAM clock-gate
            # sees activity through the scalar chain (else the next matmul
            # burst runs at 1.2 GHz)
            wk = p_sm.tile([64, 512], f32, tag="sm")
            nc.tensor.matmul(wk[:, :], tig[0:64, 0:64],
                             s_encw[0:64, 0, 0:512], start=True, stop=True,
                             tile_position=(0, 0), skip_group_check=True)
            nc.scalar.activation(tfo[64:128, :], ps_fo, AF.Tanh)
            tc16 = work.tile([128, 256], f16, tag="tc16")
            h2 = work.tile([128, 256], f16, tag="h2")
            for kk in range(2):
                ksl = slice(kk * 128, (kk + 1) * 128)
                gsl = slice(256 + kk * 128, 256 + (kk + 1) * 128)
                # u = (1+ti)*tg  on parts 0-63, fp16 (moving operand of shift)
                u16 = work.tile([64, 128], f16, tag=f"u16_{kk}")
                nc.vector.scalar_tensor_tensor(
                    u16[:, :], tig[0:64, ksl], 1.0, tig[0:64, gsl],
                    Op.add, Op.mult)
                # shift u to parts 64-127 via identity matmul
                p_ush = p_sm.tile([128, 128], f32, tag="sm")
                nc.tensor.matmul(p_ush[64:128, :], s_iden[0:64, :],
                                 u16[:, :], start=True, stop=True,
                                 tile_position=(0, 64), skip_group_check=True)
                if first:
                    # c2' = u ; tanh(c') straight from the shifted PSUM
                    nc.scalar.activation(tc16[64:128, ksl],
                                         p_ush[64:128, :], AF.Tanh, scale=0.5)
                    nc.vector.tensor_copy(c_s[64:128, ksl], p_ush[64:128, :])
                else:
                    # v = (1+tf)*c2 ; c2' = v/2 + u
                    v = work.tile([128, 128], f32, tag=f"v_{kk}")
                    nc.vector.scalar_tensor_tensor(
                        v[64:128, :], tfo[64:128, ksl], 1.0, c_s[64:128, ksl],
                        Op.add, Op.mult)
                    nc.vector.scalar_tensor_tensor(
                        c_s[64:128, ksl], v[64:128, :], 0.5, p_ush[64:128, :],
                        Op.mult, Op.add)
                    nc.scalar.activation(tc16[64:128, ksl], c_s[64:128, ksl],
                                         AF.Tanh, scale=0.5)
                # h2 = (1+to)*tanh(c')
                nc.vector.scalar_tensor_tensor(
                    h2[64:128, ksl], tfo[64:128, gsl], 1.0, tc16[64:128, ksl],
                    Op.add, Op.mult)
                pt = p_sm.tile([128, 64], f16, tag="sm")
                nc.tensor.transpose(pt[:, :], h2[64:128, ksl],
                                    s_iden[64:128, :])
                nc.vector.tensor_copy(hT_dst[:, kk, :], pt[:, :])

        def body():
            canvas = p_cv.tile([128, 32, 64], f32, tag="canvas")

            for t in range(steps):
                # ---- x_hat2 = (2x-1) - tanh(canvas/2)   [fp16, parity planes]
                if t > 0:
                    for q in range(4):
                        xh = s_xhat[:, q, :, :]
                        tcv = work.tile([128, 8, 64], f16, tag="tcv")
                        nc.scalar.activation(tcv[:, :, :],
                                             canvas[:, 8 * q:8 * (q + 1), :],
                                             AF.Tanh, scale=-0.5)
                        nc.vector.tensor_add(xh, tcv[:, :, :],
                                             s_xm1[:, q, :, :])

                # ---- encoder matmul (col-tiled halves: [i/2|g] and [f/2|o/2])
                ps_ig = p_ig.tile([64, 512], f32, tag="ig")
                ps_fo = p_fo.tile([128, 512], f32, tag="fo")
                xsrc = s_xhat if t > 0 else s_xm1
                stat = [(s_onesst[:, :], s_recw, 4)]
                if t > 0:
                    stat.append((s_hencT[:, 0, :], s_recw, 0))
                    stat.append((s_hencT[:, 1, :], s_recw, 1))
                    stat.append((s_hdecT[:, 0, :], s_recw, 2))
                    stat.append((s_hdecT[:, 1, :], s_recw, 3))
                for q in range(4):
                    for m in range(8):
                        stat.append((xsrc[:, q, m, :], s_encw, q * 8 + m))
                last = len(stat) - 1
                for j, (st, buf, jj) in enumerate(stat):
                    nc.tensor.matmul(
                        ps_ig[:, :], st, buf[:, jj, 0:512],
                        start=(j == 0), stop=(j == last),
                        tile_position=(0, 0), skip_group_check=True)
                    nc.tensor.matmul(
                        ps_fo[64:128, :], st, buf[:, jj, 512:1024],
                        start=(j == 0), stop=(j == last),
                        tile_position=(0, 64), skip_group_check=True)
                gates(ps_ig[:, :], ps_fo[64:128, :], s_cenc, s_hencT, t == 0)

                # ---- attention, batch-major softmax
                ps_lg = p_sm.tile([64, 16], f32, tag="sm")
                nc.tensor.matmul(ps_lg[:, 0:16], s_hencT[:, 0, :],
                                 s_wenc[:, 0, :], start=True, stop=False,
                                 skip_group_check=True)
                nc.tensor.matmul(ps_lg[:, 0:16], s_hencT[:, 1, :],
                                 s_wenc[:, 1, :], start=False, stop=False,
                                 skip_group_check=True)
                nc.tensor.matmul(ps_lg[:, 0:16], s_onesst[:, :],
                                 s_bencr[:, :], start=False, stop=True,
                                 skip_group_check=True)
                expv = work.tile([64, 16], f32, tag="expv")
                asum = work.tile([64, 1], f32, tag="asum")
                nc.scalar.activation(expv[:, 0:10], ps_lg[:, 0:10], AF.Exp,
                                     accum_out=asum[:, :])
                rec = work.tile([64, 1], f32, tag="rec")
                nc.vector.reciprocal(rec[:, :], asum[:, :])
                attn16 = work.tile([64, 16], f16, tag="attn16")
                nc.vector.tensor_scalar(attn16[:, 0:10], expv[:, 0:10],
                                        rec[:, :], None, Op.mult)
                pat = p_sm.tile([128, 64], f16, tag="sm")
                nc.tensor.transpose(pat[0:10, :], attn16[:, 0:10],
                                    s_iden[0:64, :])
                nc.vector.tensor_copy(s_attnT[0:10, :], pat[0:10, :])

                # ---- decoder LSTM matmul (col-tiled, K=10 attn + recurrence)
                ps_ig2 = p_ig.tile([64, 512], f32, tag="ig")
                ps_fo2 = p_fo.tile([128, 512], f32, tag="fo")
                # attn-dependent stationary LAST so the recurrence matmuls
                # overlap the attention chain
                dstat = []
                if t > 0:
                    dstat.append((s_hdecT[:, 0, :], s_decw, 1, slice(0, 128)))
                    dstat.append((s_hdecT[:, 1, :], s_decw, 2, slice(0, 128)))
                dstat.append((s_onesst[:, :], s_decw, 3, slice(0, 128)))
                dstat.append((s_attnT[0:10, :], s_decw, 0, slice(0, 10)))
                dlast = len(dstat) - 1
                for j, (st, buf, jj, ksl) in enumerate(dstat):
                    nc.tensor.matmul(
                        ps_ig2[:, :], st, buf[ksl, jj, 0:512],
                        start=(j == 0), stop=(j == dlast),
                        tile_position=(0, 0), skip_group_check=True)
                    nc.tensor.matmul(
                        ps_fo2[64:128, :], st, buf[ksl, jj, 512:1024],
                        start=(j == 0), stop=(j == dlast),
                        tile_position=(0, 64), skip_group_check=True)
                gates(ps_ig2[:, :], ps_fo2[64:128, :], s_cdec, s_hdecT, t == 0)

                # ---- canvas += h2_dec @ (W_dec/2)  (PSUM-resident accumulation)
                for q in range(4):
                    for k in range(2):
                        for m in range(8 * q, 8 * (q + 1)):
                            nc.tensor.matmul(
                                canvas[:, m, :],
                                s_wdec[:, k, m * 128:(m + 1) * 128],
                                s_hdecT[:, k, :],
                                start=(t == 0 and k == 0 and m % 8 == 0),
                                stop=(t == steps - 1 and k == 1
                                      and not has_bdec),
                                skip_group_check=True)
                if has_bdec:
                    for m in range(32):
                        nc.tensor.matmul(
                            canvas[:, m, :],
                            s_bdec[0:1, m * 128:(m + 1) * 128],
                            s_onesst[0:1, 0:64],
                            start=False,
                            stop=(t == steps - 1 and m == 31),
                            skip_group_check=True)

            # evacuate canvas PSUM -> SBUF -> DRAM: static buffer (no pool
            # recycling stalls), copies alternate Scalar/Vector, DMAs spread
            # over four queues
            s_out = nc.alloc_sbuf_tensor("s_out", [128, 2048], f32)
            dmaq = [nc.sync, nc.gpsimd, nc.scalar]
            for m4 in range(8):
                cv = s_out[:, m4 * 256:(m4 + 1) * 256]
                src = canvas[:, m4 * 4:(m4 + 1) * 4, :]
                if m4 % 2 == 0:
                    nc.vector.tensor_copy(cv, src)
                else:
                    nc.scalar.activation(cv, src, AF.Copy)
                dmaq[m4 % 3].dma_start(
                    out=d_out.ap()[:, m4 * 256:(m4 + 1) * 256], in_=cv)

        body()
        ctx.close()

    nc.compile()
    return nc


# ---------------------------------------------------------------- host packing
def host_pack(inputs):
    """Preprocess full inputs -> (shared weight map, per-core input maps)."""
    f16 = np.float16
    ek = np.asarray(inputs["enc_kernel"], np.float32)
    # xhat2 = 2*xhat fold: A/2 ; gate permutation/scaling via perm_scale
    A = perm_scale(fold_enc_kernel(ek[:9216])[PIX] * 0.5)    # [4096, 1024]
    hdf = perm_scale((ek[9216:9472] + ek[9472:9728]) * 0.5)  # h2 fold
    enc_rec = perm_scale(np.asarray(inputs["enc_rec"], np.float32) * 0.5)
    enc_bias = np.asarray(inputs["enc_bias"], np.float32)
    dec_rec = perm_scale(np.asarray(inputs["dec_rec"], np.float32) * 0.5)
    dec_bias = np.asarray(inputs["dec_bias"], np.float32)
    W_enc = np.asarray(inputs["W_enc"], np.float32)
    b_enc = np.asarray(inputs["b_enc"], np.float32)
    W_dec = np.asarray(inputs["W_dec"], np.float32)
    b_dec = np.asarray(inputs["b_dec"], np.float32)
    dec_k = np.asarray(inputs["dec_kernel"], np.float32)
    Wz = perm_scale(W_enc.T @ dec_k)                         # [10, 1024]

    encw = A.reshape(32, 128, 1024).transpose(1, 0, 2).reshape(128, -1)

    def brow(bias1024):
        t = np.zeros((128, 1024), np.float32)
        t[0] = bias1024
        return t

    recw = np.stack([enc_rec[0:128], enc_rec[128:256], hdf[0:128], hdf[128:256],
                     brow(perm_scale(enc_bias[None, :])[0])]
                    ).transpose(1, 0, 2).reshape(128, -1)
    wz_blk = np.zeros((128, 1024), np.float32)
    wz_blk[0:10] = Wz
    decw = np.stack([wz_blk, dec_rec[0:128], dec_rec[128:256],
                     brow(perm_scale(dec_bias[None, :])[0])]
                    ).transpose(1, 0, 2).reshape(128, -1)
    # h2 fold for canvas: W_dec/2
    wdec = (W_dec[:, PIX] * 0.5).reshape(2, 128, 4096).transpose(1, 0, 2
                                                                 ).reshape(128, -1)
    # logits weights: W_enc/2 (h2 fold), [256,10] -> two [128,16] chunks
    wenc = np.zeros((128, 2, 16), np.float32)
    wenc[:, 0, 0:10] = W_enc[0:128] * 0.5
    wenc[:, 1, 0:10] = W_enc[128:256] * 0.5
    bencr = np.zeros((128, 16), np.float32)
    bencr[0, 0:10] = b_enc
    iden = np.zeros((128, 64), np.float32)
    iden[0:64] = np.eye(64)
    iden[64:128] = np.eye(64)
    onesst = np.zeros((128, 64), np.float32)
    onesst[0] = 1.0

    shared = {
        "encw": encw.astype(f16), "recw": recw.astype(f16),
        "decw": decw.astype(f16), "wdec": wdec.astype(f16),
        "wenc": wenc.reshape(128, 32).astype(f16),
        "bencr": bencr.astype(f16),
        "iden": iden.astype(f16),
        "onesst": onesst.astype(f16),
    }
    has_bdec = bool(np.any(b_dec))
    if has_bdec:
        shared["bdec"] = b_dec[PIX].reshape(1, 4096).astype(f16)

    x = np.asarray(inputs["x"], np.float32)
    B = x.shape[0]
    assert B == NCORES * BL, f"expected batch {NCORES * BL}, got {B}"
    in_maps = []
    for c in range(NCORES):
        xc = 2.0 * x[c * BL:(c + 1) * BL].reshape(BL, 4096).T[PIX] - 1.0
        xm1 = (xc.reshape(4, 8, 128, BL).transpose(2, 0, 1, 3)
               .reshape(128, 2048).astype(f16))
        m = dict(shared)
        m["xm1"] = xm1
        in_maps.append(m)
    return in_maps, has_bdec


def unpack_output(res_core):
    """[128, 2048] f32 parity-major canvas^T -> [BL, 64, 64]."""
    buf = res_core.reshape(128, 32, 64).transpose(1, 0, 2).reshape(4096, BL)
    out = np.empty((4096, BL), np.float32)
    out[PIX] = buf
    return out.T.reshape(BL, IMG, IMG)


_NC_CACHE = {}


def _get_nc(has_bdec=False):
    key = (has_bdec,)
    if key not in _NC_CACHE:
        _NC_CACHE[key] = build_program(has_bdec=has_bdec)
    return _NC_CACHE[key]


def kernel(**inputs):
    import sys
    if "/opt/trn_rl_repo" not in sys.path:
        sys.path.insert(0, "/opt/trn_rl_repo")
    from concourse import bass_utils

    in_maps, has_bdec = host_pack(inputs)
    nc = _get_nc(has_bdec=has_bdec)
    res = bass_utils.run_bass_kernel_spmd(nc, in_maps,
                                          core_ids=list(range(NCORES)))
    outs = [unpack_output(np.asarray(res.results[c]["canvas"]))
            for c in range(NCORES)]
    return np.concatenate(outs, axis=0).astype(
        np.asarray(inputs["x"]).dtype, copy=False)


# revision 16
# speedup vs baseline: 1.1122x; 1.1122x over previous
"""DRAW-model Trainium2 kernel (8 NeuronCores, data-parallel over batch).

Strategy
--------
Pure data parallelism: 8 cores x 64 local batch, zero collectives.  All
weights SBUF-resident fp16; activations feature-minor ("transposed") so the
big encoder matmul streams N=512-wide moving operands at full PE rate.

v2 changes over the original baseline:
  * all sigmoids expressed as tanh (sig(x) = (1+tanh(x/2))/2) with the
    0.5 factors folded into host-side weights (cell state kept as c2=2c,
    hidden state as h2=2h with halved downstream weights).  Every ACT
    function used (Tanh, Exp) then lives in the single `exp_and_others`
    activation table -> zero per-step ACT_TABLE_LOADs (was 2 x 1.3us).
  * gate columns permuted to [i/2 | g] ++ [f/2 | o/2] so each col-tiled
    matmul half needs exactly one N=512 tanh; the cross-half join
    (c2' = v/2 + u) uses a PE identity-shift matmul instead of the
    per-step SBUF->SBUF DMA.
  * attention reformulated batch-major: logits [64,10] -> Exp with
    accum_out (fused row-sum) -> reciprocal -> per-partition scale ->
    one PE transpose.  The decoder consumes attn directly through the
    host-folded Wz = W_enc^T @ dec_kernel [10,1024], killing the old
    z-materialization (2 matmuls + broadcast matmul + 2 vector muls).
  * decoder matmul col-tiled like the encoder (concurrent halves).
  * dependency-anchored LDWEIGHTS heartbeats inside the gate chains keep
    the PE HAM clock-gate at K=8/8 (the old kernel re-throttled to
    1.2 GHz every step during the ~5us scalar window, running the
    decoder+canvas matmuls at half clock).
  * PSUM evacuation copies moved from the Scalar to the Vector engine.
"""

import numpy as np

STEPS = 10
UNITS = 256
BL = 64          # local batch per core
NCORES = 8
IMG = 64

# ---------------------------------------------------------------- host index math
def _pix_order():
    # new pixel index n = q*1024 + j*32 + i  ->  original pixel (2j+pr)*64 + (2i+pc)
    # with q = pr*2 + pc
    out = np.empty(4096, np.int64)
    n = 0
    for pr in range(2):
        for pc in range(2):
            for j in range(32):
                for i in range(32):
                    out[n] = (2 * j + pr) * 64 + (2 * i + pc)
                    n += 1
    return out


PIX = _pix_order()


def fold_enc_kernel(W):
    """Collapse extract_patches into the weight: each patch feature copies one
    pixel of x_hat, so patches @ W[:9216] == x_hat_flat @ A with
    A[p,:] = sum of W rows whose feature reads pixel p.  [4096, 1024]."""
    A = np.zeros((4096, W.shape[1]), np.float32)
    r_idx = np.arange(32)
    for dy in range(3):
        rows = 2 * r_idx + dy
        rv = r_idx[rows < 64]
        for dx in range(3):
            cols = 2 * r_idx + dx
            cv = r_idx[cols < 64]
            pix = (2 * rv[:, None] + dy) * 64 + (2 * cv[None, :] + dx)
            feat = (rv[:, None] * 32 + cv[None, :]) * 9 + (dy * 3 + dx)
            A[pix.ravel()] += W[feat.ravel()]
    return A


def perm_scale(W):
    """Permute gate columns i,f,g,o -> [i/2, g, f/2, o/2] so that the
    (0,0) col-tile half computes [z_i/2 | z_g] and the (0,64) half
    [z_f/2 | z_o/2]; tanh of those gives ti,tg,tf,to directly."""
    return np.concatenate([W[:, 0:256] * 0.5, W[:, 512:768],
                           W[:, 256:512] * 0.5, W[:, 768:1024] * 0.5], axis=1)


# ---------------------------------------------------------------- program builder
def build_program(steps=STEPS, has_bdec=False):
    """Build + compile the per-core Bass program.  Returns nc."""
    import concourse.bacc as bacc
    import concourse.tile as tile
    import concourse.mybir as mybir
    from concourse.alu_op_type import AluOpType as Op

    f16 = mybir.dt.float16
    f32 = mybir.dt.float32
    AF = mybir.ActivationFunctionType

    nc = bacc.Bacc("TRN2", target_bir_lowering=False, debug=False,
                   dynamic_dma_scratch_size=2048)

    def din(name, shape, dt):
        return nc.dram_tensor(name, shape, dt, kind="ExternalInput")

    d_encw = din("encw", [128, 32 * 1024], f16)
    d_recw = din("recw", [128, 5 * 1024], f16)
    d_decw = din("decw", [128, 4 * 1024], f16)
    d_wdec = din("wdec", [128, 2 * 4096], f16)
    d_wenc = din("wenc", [128, 32], f16)
    d_bencr = din("bencr", [128, 16], f16)
    d_iden = din("iden", [128, 64], f16)
    d_onesst = din("onesst", [128, 64], f16)
    d_xm1 = din("xm1", [128, 2048], f16)
    if has_bdec:
        d_bdec = din("bdec", [1, 4096], f16)
    d_out = nc.dram_tensor("canvas", [128, 2048], f32, kind="ExternalOutput")

    with tile.TileContext(nc) as tc:
        # ---------------- static SBUF
        s_encw = nc.alloc_sbuf_tensor("s_encw", [128, 32, 1024], f16)
        s_recw = nc.alloc_sbuf_tensor("s_recw", [128, 5, 1024], f16)
        s_decw = nc.alloc_sbuf_tensor("s_decw", [128, 4, 1024], f16)
        s_wdec = nc.alloc_sbuf_tensor("s_wdec", [128, 2, 4096], f16)
        s_wenc = nc.alloc_sbuf_tensor("s_wenc", [128, 2, 16], f16)
        s_bencr = nc.alloc_sbuf_tensor("s_bencr", [128, 16], f16)
        s_iden = nc.alloc_sbuf_tensor("s_iden", [128, 64], f16)
        s_onesst = nc.alloc_sbuf_tensor("s_onesst", [128, 64], f16)
        s_xm1 = nc.alloc_sbuf_tensor("s_xm1", [128, 4, 8, 64], f16)
        s_xhat = nc.alloc_sbuf_tensor("s_xhat", [128, 4, 8, 64], f16)
        s_hencT = nc.alloc_sbuf_tensor("s_hencT", [128, 2, 64], f16)
        s_hdecT = nc.alloc_sbuf_tensor("s_hdecT", [128, 2, 64], f16)
        s_attnT = nc.alloc_sbuf_tensor("s_attnT", [16, 64], f16)
        # cell states live on partitions 64-127 (with the f|o gate half)
        s_cenc = nc.alloc_sbuf_tensor("s_cenc", [128, 256], f32)
        s_cdec = nc.alloc_sbuf_tensor("s_cdec", [128, 256], f32)
        if has_bdec:
            s_bdec = nc.alloc_sbuf_tensor("s_bdec", [1, 4096], f16)

        # ---------------- load weights / constants (two parallel DMA queues:
        # sync carries xm1 + encw in consumption order; scalar carries the
        # small weights + decoder-side tensors needed mid-step-0)
        recw_d_ap = d_recw.ap().rearrange("p (t n) -> p t n", n=1024)
        for dst, src in [
            (s_onesst[:, :], d_onesst.ap()),
            (s_xm1[:, :, :, :], d_xm1.ap()),
        ]:
            nc.sync.dma_start(out=dst, in_=src)
        for dst, src in [
            (s_recw[:, 4, :], recw_d_ap[:, 4, :]),   # bias row: 1st matmul dep
            (s_iden[:, :], d_iden.ap()),
            (s_recw[:, 0:4, :], recw_d_ap[:, 0:4, :]),
            (s_wenc[:, :, :], d_wenc.ap()), (s_bencr[:, :], d_bencr.ap()),
            (s_decw[:, :, :], d_decw.ap()), (s_wdec[:, :, :], d_wdec.ap()),
        ]:
            nc.scalar.dma_start(out=dst, in_=src)
        encw_d_ap = d_encw.ap().rearrange("p (t n) -> p t n", n=1024)
        for g in range(16):
            sl = slice(g * 2, (g + 1) * 2)
            nc.sync.dma_start(out=s_encw[:, sl, :], in_=encw_d_ap[:, sl, :])
        if has_bdec:
            nc.scalar.dma_start(out=s_bdec[:, :], in_=d_bdec.ap())

        # ---------------- pools
        import contextlib
        ctx = contextlib.ExitStack()
        work = ctx.enter_context(tc.tile_pool(name="work", bufs=2))
        p_ig = ctx.enter_context(tc.tile_pool(name="p_ig", bufs=1, space="PSUM"))
        p_fo = ctx.enter_context(tc.tile_pool(name="p_fo", bufs=1, space="PSUM"))
        p_sm = ctx.enter_context(tc.tile_pool(name="p_sm", bufs=2, space="PSUM"))
        p_cv = ctx.enter_context(tc.tile_pool(name="p_cv", bufs=1, space="PSUM"))

        def gates(ps_ig, ps_fo, c_s, hT_dst, first):
            """Tanh-only LSTM gate math, pipelined in two 128-col halves.
            ps_ig: [64,512] PSUM AP holding [z_i/2 | z_g] on parts 0-63;
            ps_fo: [64,512] AP on parts 64-127 holding [z_f/2 | z_o/2].
            c_s: [64,256] f32 SBUF at parts 64-127 (c2 = 2c).  Writes
            transposed fp16 h2 into hT_dst."""
            tig = work.tile([128, 512], f16, tag="tig")
            tfo = work.tile([128, 512], f16, tag="tfo")
            nc.scalar.activation(tig[0:64, :], ps_ig, AF.Tanh)
            # warm-keeper: dependency-anchored PE work so the HAM clock-gate
            # sees activity through the scalar chain (else the next matmul
            # burst runs at 1.2 GHz)
            wk = p_sm.tile([128, 64], f16, tag="sm")
            nc.tensor.transpose(wk[:, :], tig[0:64, 0:128], s_iden[0:64, :])
            nc.scalar.activation(tfo[64:128, :], ps_fo, AF.Tanh)
            # u = (1+ti)*tg  on parts 0-63, fp16 (moving operand of shift)
            u16 = work.tile([64, 256], f16, tag="u16")
            nc.vector.scalar_tensor_tensor(
                u16[:, :], tig[0:64, 0:256], 1.0, tig[0:64, 256:512],
                Op.add, Op.mult)
            # shift u to parts 64-127 via identity matmul
            p_ush = p_sm.tile([128, 256], f32, tag="sm")
            nc.tensor.matmul(p_ush[64:128, :], s_iden[0:64, :], u16[:, :],
                             start=True, stop=True, tile_position=(0, 64),
                             skip_group_check=True)
            tc16 = work.tile([128, 256], f16, tag="tc16")
            if first:
                # c2' = u ; tanh(c') straight from the shifted PSUM
                nc.scalar.activation(tc16[64:128, :], p_ush[64:128, :],
                                     AF.Tanh, scale=0.5)
                nc.vector.tensor_copy(c_s[64:128, :], p_ush[64:128, :])
            else:
                # v = (1+tf)*c2 ; c2' = v/2 + u
                v = work.tile([128, 256], f32, tag="v")
                nc.vector.scalar_tensor_tensor(
                    v[64:128, :], tfo[64:128, 0:256], 1.0, c_s[64:128, :],
                    Op.add, Op.mult)
                nc.vector.scalar_tensor_tensor(
                    c_s[64:128, :], v[64:128, :], 0.5, p_ush[64:128, :],
                    Op.mult, Op.add)
                nc.scalar.activation(tc16[64:128, :], c_s[64:128, :],
                                     AF.Tanh, scale=0.5)
            wk2 = p_sm.tile([128, 64], f16, tag="sm")
            nc.tensor.transpose(wk2[:, :], tc16[64:128, 0:128],
                                s_iden[64:128, :])
            # h2 = (1+to)*tanh(c')
            h2 = work.tile([128, 256], f16, tag="h2")
            nc.vector.scalar_tensor_tensor(
                h2[64:128, :], tfo[64:128, 256:512], 1.0, tc16[64:128, :],
                Op.add, Op.mult)
            for k in range(2):
                pt = p_sm.tile([128, 64], f16, tag="sm")
                nc.tensor.transpose(pt[:, :], h2[64:128, k * 128:(k + 1) * 128],
                                    s_iden[64:128, :])
                nc.vector.tensor_copy(hT_dst[:, k, :], pt[:, :])

        def body():
            canvas = p_cv.tile([128, 32, 64], f32, tag="canvas")

            for t in range(steps):
                # ---- x_hat2 = (2x-1) - tanh(canvas/2)   [fp16, parity planes]
                if t > 0:
                    for q in range(4):
                        xh = s_xhat[:, q, :, :]
                        tcv = work.tile([128, 8, 64], f16, tag="tcv")
                        nc.scalar.activation(tcv[:, :, :],
                                             canvas[:, 8 * q:8 * (q + 1), :],
                                             AF.Tanh, scale=-0.5)
                        nc.vector.tensor_add(xh, tcv[:, :, :],
                                             s_xm1[:, q, :, :])

                # ---- encoder matmul (col-tiled halves: [i/2|g] and [f/2|o/2])
                ps_ig = p_ig.tile([64, 512], f32, tag="ig")
                ps_fo = p_fo.tile([128, 512], f32, tag="fo")
                xsrc = s_xhat if t > 0 else s_xm1
                stat = [(s_onesst[:, :], s_recw, 4)]
                if t > 0:
                    stat.append((s_hencT[:, 0, :], s_recw, 0))
                    stat.append((s_hencT[:, 1, :], s_recw, 1))
                    stat.append((s_hdecT[:, 0, :], s_recw, 2))
                    stat.append((s_hdecT[:, 1, :], s_recw, 3))
                for q in range(4):
                    for m in range(8):
                        stat.append((xsrc[:, q, m, :], s_encw, q * 8 + m))
                last = len(stat) - 1
                for j, (st, buf, jj) in enumerate(stat):
                    nc.tensor.matmul(
                        ps_ig[:, :], st, buf[:, jj, 0:512],
                        start=(j == 0), stop=(j == last),
                        tile_position=(0, 0), skip_group_check=True)
                    nc.tensor.matmul(
                        ps_fo[64:128, :], st, buf[:, jj, 512:1024],
                        start=(j == 0), stop=(j == last),
                        tile_position=(0, 64), skip_group_check=True)
                gates(ps_ig[:, :], ps_fo[64:128, :], s_cenc, s_hencT, t == 0)

                # ---- attention, batch-major softmax
                ps_lg = p_sm.tile([64, 16], f32, tag="sm")
                nc.tensor.matmul(ps_lg[:, 0:16], s_hencT[:, 0, :],
                                 s_wenc[:, 0, :], start=True, stop=False,
                                 skip_group_check=True)
                nc.tensor.matmul(ps_lg[:, 0:16], s_hencT[:, 1, :],
                                 s_wenc[:, 1, :], start=False, stop=False,
                                 skip_group_check=True)
                nc.tensor.matmul(ps_lg[:, 0:16], s_onesst[:, :],
                                 s_bencr[:, :], start=False, stop=True,
                                 skip_group_check=True)
                expv = work.tile([64, 16], f32, tag="expv")
                asum = work.tile([64, 1], f32, tag="asum")
                nc.scalar.activation(expv[:, 0:10], ps_lg[:, 0:10], AF.Exp,
                                     accum_out=asum[:, :])
                rec = work.tile([64, 1], f32, tag="rec")
                nc.vector.reciprocal(rec[:, :], asum[:, :])
                attn16 = work.tile([64, 16], f16, tag="attn16")
                nc.vector.tensor_scalar(attn16[:, 0:10], expv[:, 0:10],
                                        rec[:, :], None, Op.mult)
                pat = p_sm.tile([128, 64], f16, tag="sm")
                nc.tensor.transpose(pat[0:10, :], attn16[:, 0:10],
                                    s_iden[0:64, :])
                nc.vector.tensor_copy(s_attnT[0:10, :], pat[0:10, :])

                # ---- decoder LSTM matmul (col-tiled, K=10 attn + recurrence)
                ps_ig2 = p_ig.tile([64, 512], f32, tag="ig")
                ps_fo2 = p_fo.tile([128, 512], f32, tag="fo")
                # attn-dependent stationary LAST so the recurrence matmuls
                # overlap the attention chain
                dstat = []
                if t > 0:
                    dstat.append((s_hdecT[:, 0, :], s_decw, 1, slice(0, 128)))
                    dstat.append((s_hdecT[:, 1, :], s_decw, 2, slice(0, 128)))
                dstat.append((s_onesst[:, :], s_decw, 3, slice(0, 128)))
                dstat.append((s_attnT[0:10, :], s_decw, 0, slice(0, 10)))
                dlast = len(dstat) - 1
                for j, (st, buf, jj, ksl) in enumerate(dstat):
                    nc.tensor.matmul(
                        ps_ig2[:, :], st, buf[ksl, jj, 0:512],
                        start=(j == 0), stop=(j == dlast),
                        tile_position=(0, 0), skip_group_check=True)
                    nc.tensor.matmul(
                        ps_fo2[64:128, :], st, buf[ksl, jj, 512:1024],
                        start=(j == 0), stop=(j == dlast),
                        tile_position=(0, 64), skip_group_check=True)
                gates(ps_ig2[:, :], ps_fo2[64:128, :], s_cdec, s_hdecT, t == 0)

                # ---- canvas += h2_dec @ (W_dec/2)  (PSUM-resident accumulation)
                for q in range(4):
                    for k in range(2):
                        for m in range(8 * q, 8 * (q + 1)):
                            nc.tensor.matmul(
                                canvas[:, m, :],
                                s_wdec[:, k, m * 128:(m + 1) * 128],
                                s_hdecT[:, k, :],
                                start=(t == 0 and k == 0 and m % 8 == 0),
                                stop=(t == steps - 1 and k == 1
                                      and not has_bdec),
                                skip_group_check=True)
                if has_bdec:
                    for m in range(32):
                        nc.tensor.matmul(
                            canvas[:, m, :],
                            s_bdec[0:1, m * 128:(m + 1) * 128],
                            s_onesst[0:1, 0:64],
                            start=False,
                            stop=(t == steps - 1 and m == 31),
                            skip_group_check=True)

            # evacuate canvas PSUM -> SBUF -> DRAM: static buffer (no pool
            # recycling stalls), copies alternate Scalar/Vector, DMAs spread
            # over four queues
            s_out = nc.alloc_sbuf_tensor("s_out", [128, 2048], f32)
            dmaq = [nc.sync, nc.gpsimd, nc.scalar]
            for m4 in range(8):
                cv = s_out[:, m4 * 256:(m4 + 1) * 256]
                src = canvas[:, m4 * 4:(m4 + 1) * 4, :]
                if m4 % 2 == 0:
                    nc.vector.tensor_copy(cv, src)
                else:
                    nc.scalar.activation(cv, src, AF.Copy)
                dmaq[m4 % 3].dma_start(
                    out=d_out.ap()[:, m4 * 256:(m4 + 1) * 256], in_=cv)

        body()
        ctx.close()

    nc.compile()
    return nc


# ---------------------------------------------------------------- host packing
def host_pack(inputs):
    """Preprocess full inputs -> (shared weight map, per-core input maps)."""
    f16 = np.float16
    ek = np.asarray(inputs["enc_kernel"], np.float32)
    # xhat2 = 2*xhat fold: A/2 ; gate permutation/scaling via perm_scale
    A = perm_scale(fold_enc_kernel(ek[:9216])[PIX] * 0.5)    # [4096, 1024]
    hdf = perm_scale((ek[9216:9472] + ek[9472:9728]) * 0.5)  # h2 fold
    enc_rec = perm_scale(np.asarray(inputs["enc_rec"], np.float32) * 0.5)
    enc_bias = np.asarray(inputs["enc_bias"], np.float32)
    dec_rec = perm_scale(np.asarray(inputs["dec_rec"], np.float32) * 0.5)
    dec_bias = np.asarray(inputs["dec_bias"], np.float32)
    W_enc = np.asarray(inputs["W_enc"], np.float32)
    b_enc = np.asarray(inputs["b_enc"], np.float32)
    W_dec = np.asarray(inputs["W_dec"], np.float32)
    b_dec = np.asarray(inputs["b_dec"], np.float32)
    dec_k = np.asarray(inputs["dec_kernel"], np.float32)
    Wz = perm_scale(W_enc.T @ dec_k)                         # [10, 1024]

    encw = A.reshape(32, 128, 1024).transpose(1, 0, 2).reshape(128, -1)

    def brow(bias1024):
        t = np.zeros((128, 1024), np.float32)
        t[0] = bias1024
        return t

    recw = np.stack([enc_rec[0:128], enc_rec[128:256], hdf[0:128], hdf[128:256],
                     brow(perm_scale(enc_bias[None, :])[0])]
                    ).transpose(1, 0, 2).reshape(128, -1)
    wz_blk = np.zeros((128, 1024), np.float32)
    wz_blk[0:10] = Wz
    decw = np.stack([wz_blk, dec_rec[0:128], dec_rec[128:256],
                     brow(perm_scale(dec_bias[None, :])[0])]
                    ).transpose(1, 0, 2).reshape(128, -1)
    # h2 fold for canvas: W_dec/2
    wdec = (W_dec[:, PIX] * 0.5).reshape(2, 128, 4096).transpose(1, 0, 2
                                                                 ).reshape(128, -1)
    # logits weights: W_enc/2 (h2 fold), [256,10] -> two [128,16] chunks
    wenc = np.zeros((128, 2, 16), np.float32)
    wenc[:, 0, 0:10] = W_enc[0:128] * 0.5
    wenc[:, 1, 0:10] = W_enc[128:256] * 0.5
    bencr = np.zeros((128, 16), np.float32)
    bencr[0, 0:10] = b_enc
    iden = np.zeros((128, 64), np.float32)
    iden[0:64] = np.eye(64)
    iden[64:128] = np.eye(64)
    onesst = np.zeros((128, 64), np.float32)
    onesst[0] = 1.0

    shared = {
        "encw": encw.astype(f16), "recw": recw.astype(f16),
        "decw": decw.astype(f16), "wdec": wdec.astype(f16),
        "wenc": wenc.reshape(128, 32).astype(f16),
        "bencr": bencr.astype(f16),
        "iden": iden.astype(f16),
        "onesst": onesst.astype(f16),
    }
    has_bdec = bool(np.any(b_dec))
    if has_bdec:
        shared["bdec"] = b_dec[PIX].reshape(1, 4096).astype(f16)

    x = np.asarray(inputs["x"], np.float32)
    B = x.shape[0]
    assert B == NCORES * BL, f"expected batch {NCORES * BL}, got {B}"
    in_maps = []
    for c in range(NCORES):
        xc = 2.0 * x[c * BL:(c + 1) * BL].reshape(BL, 4096).T[PIX] - 1.0
        xm1 = (xc.reshape(4, 8, 128, BL).transpose(2, 0, 1, 3)
               .reshape(128, 2048).astype(f16))
        m = dict(shared)
        m["xm1"] = xm1
        in_maps.append(m)
    return in_maps, has_bdec


def unpack_output(res_core):
    """[128, 2048] f32 parity-major canvas^T -> [BL, 64, 64]."""
    buf = res_core.reshape(128, 32, 64).transpose(1, 0, 2).reshape(4096, BL)
    out = np.empty((4096, BL), np.float32)
    out[PIX] = buf
    return out.T.reshape(BL, IMG, IMG)


_NC_CACHE = {}


def _get_nc(has_bdec=False):
    key = (has_bdec,)
    if key not in _NC_CACHE:
        _NC_CACHE[key] = build_program(has_bdec=has_bdec)
    return _NC_CACHE[key]


def kernel(**inputs):
    import sys
    if "/opt/trn_rl_repo" not in sys.path:
        sys.path.insert(0, "/opt/trn_rl_repo")
    from concourse import bass_utils

    in_maps, has_bdec = host_pack(inputs)
    nc = _get_nc(has_bdec=has_bdec)
    res = bass_utils.run_bass_kernel_spmd(nc, in_maps,
                                          core_ids=list(range(NCORES)))
    outs = [unpack_output(np.asarray(res.results[c]["canvas"]))
            for c in range(NCORES)]
    return np.concatenate(outs, axis=0).astype(
        np.asarray(inputs["x"]).dtype, copy=False)
